# revision 78
# baseline (speedup 1.0000x reference)
"""CASSI GAP reconstruction (DifferentiableGAPTV) on 8 Trainium2 NeuronCores.

Strategy: shard H=512 rows across 8 cores as 128-row slabs (64 output rows +
32-row halo each side).  Rows are independent except the depthwise conv
(3-tap => +-1 row/iter * 12 iters = 12-row dependency), so the halo makes the
whole 12-iteration loop collective-free; each core's central 64 rows are exact.

Numerics (validated vs the fp32 reference on CPU, rel err ~1e-2 < 2e-2):
 - 5-tap sigma=0.5 Gaussian -> renormalized 3-tap (outer taps are 2.6e-4).
 - band states, masks, and per-band elementwise ops in bf16 (DVE 2x mode);
   the measurement-plane accumulator s = y1 + y stays fp32 (updated via
   s' = 0.5*(s + t0) + y since t0 = s - 2*yb, so GPSIMD never touches PSUM).

Per-core, per iteration (bands l = 0..27, dx[l] == l):
  A:  yb = sum_l shift_l(m*x_l)  -- identity matmuls into a PSUM plane, all
      emitted at the END of the iteration (their inputs u_l = m*x_l trickle
      in from DVE/GPSIMD as the copies land); the boundary chain is then
      just "last A-matmul -> t0" and the conv(x) matmuls of the next
      iteration's split bands keep PE busy across it.
  B:  t0 = s - 2*yb (DVE stt, bf16 out)
  C:  v_l = mi_l * t0[l:l+W]  (mi = m/Phi_sum, host-precomputed bf16;
      one DVE op per 4 bands via an overlapping-window AP, dx[l] == l)
      bands 0,1:   x_l' = conv(x_l) + conv(v_l)  (6 matmuls, no w needed)
      bands 2..27: w_l = x_l + v_l (DVE, batched, two quads ahead of PE),
        x_l' = conv(w_l)
      conv = 2D 3x3 via 3 matmuls (row conv in the weights, col taps as
      +-1-shifted rhs windows).  For WING_QUADS the two +-1 col taps are
      folded into one matmul on sw = w<<1 + w>>1 (g3[0] == g3[2]), trading
      PE time for DVE time per band.
      Conv outputs land in [128,2,W] PSUM pair tiles; PSUM->SBUF copies run
      on ACT one PAIR at a time (halves the per-op PSUM access overhead);
      on the last iteration each pair stages through an fp32 SBUF tile and
      DMAs to DRAM with a permuted out-AP (one DMA per band pair).
  Scheduling: the s update runs at the head of the Pool queue (add on
      GPSIMD, 0.5-scale on ACT, y-add deferred); A(k+1) matmuls interleave
      into the conv loop lagging ~3 quads; u pair 13 is computed before 12
      on DVE so the A group's stop matmul never waits on DVE's last op.
"""
import sys

sys.path.insert(0, "/opt/trn_rl_repo")
import numpy as np
import ml_dtypes
import concourse.bass as bass
import concourse.mybir as mybir
import concourse.tile as tile
from concourse.bass_utils import run_bass_kernel_spmd
from bass_rust import AP

H, W, L = 512, 512, 28
N_ITER = 12
SIGMA = 0.5
PI = 3.141592653589793
NCORES = 8
ROWS = 128          # slab rows per core
OUT_ROWS = 64       # exact output rows per core
HALO = 32           # (ROWS - OUT_ROWS) / 2
WM = W + L - 1      # measurement-plane width (539)
XP = W + 4          # padded band pitch (2 zero cols each side)

f32 = mybir.dt.float32
bf16 = mybir.dt.bfloat16
fp8 = mybir.dt.float8e4
DRMODE = mybir.MatmulPerfMode.DoubleRow
MUL = mybir.AluOpType.mult
ADD = mybir.AluOpType.add

NPAIR = L // 2                     # 14 band pairs
POOL_U_PAIRS = tuple(range(8))     # u pairs computed on GPSIMD
WING_QUADS = (4, 5)                # quads using the sw wing-sum conv
# fp8-DoubleRow A: the A(1..FP8_LAST_A) scatter matmuls run in fp8e4 with
# perf_mode=DoubleRow (one matmul accumulates a PRE-SHIFTED band pair at 0.5
# cyc/col).  GAP self-corrects: with >=4 exact tail iterations the final
# rel err is unchanged (CPU sim: 8.0e-3 vs 7.9e-3 all-bf16).
FP8_LAST_A = 0   # fp8-DR A disabled: A-block is off the critical path, and the
                 # freed PE time cannot shorten the DVE-bound cycle (measured
                 # 350us vs 327us); code path kept for future use
PW8 = 528                          # fp8 u-pair band pitch (16B-aligned)


def _offsets(s, phi_deg):
    phi = phi_deg * PI / 180.0
    dx = s * np.cos(phi)
    dy = s * np.sin(phi)
    dx = dx - dx.min()
    dy = dy - dy.min()
    return np.rint(dx).astype(np.int32), np.rint(dy).astype(np.int32)


def _gauss3(sigma):
    ksize = max(3, int(6 * sigma + 1) | 1)
    ax = np.arange(ksize, dtype=np.float32) - ksize // 2
    g1 = np.exp(-0.5 * (ax / sigma) ** 2)
    g1 = g1 / g1.sum()
    c = ksize // 2
    g3 = g1[c - 1 : c + 2].astype(np.float64)
    g3 = (g3 / g3.sum()).astype(np.float32)
    return g3  # [3]


def _split_excess_waits(nc, max_w=1):
    """walrus in this toolchain accepts at most one sync wait per instruction;
    hoist excess waits onto preceding same-engine NoOp carriers."""
    ctr = 0
    for f in nc.m.functions:
        for bb in f.blocks:
            il = bb.instructions
            i = 0
            while i < len(il):
                inst = il[i]
                si = inst.sync_info
                w = list(si.on_wait) if (si and si.on_wait) else []
                if len(w) > max_w:
                    si.on_wait = w[-max_w:]
                    extra = w[:-max_w]
                    pos = i
                    for j in range(0, len(extra), max_w):
                        ctr += 1
                        nop = mybir.InstNoOp(
                            name=f"I-waitsplit-{ctr}", ins=[], outs=[]
                        )
                        nop.engine = inst.engine
                        nop.sync_info = mybir.SyncInfo(
                            on_wait=extra[j : j + max_w], on_update=[]
                        )
                        il.insert(pos, nop)
                        pos += 1
                        i += 1
                i += 1


def _win3(tile2d, l0, n, w):
    """[128, n, w] overlapping-window view of a [128, >=l0+n-1+w] tile:
    out[:, j, c] = tile2d[:, l0 + j + c]  (band axis stride = 1 element)."""
    base = tile2d[:, l0 : l0 + w]
    pairs = [list(p) for p in base.ap]
    assert len(pairs) == 2
    return AP(base.tensor, base.offset, [pairs[0], [1, n], [1, w]])


def build_nc(n_iter=N_ITER):
    nc = bass.Bass()
    y_in = nc.declare_dram_parameter("y_slab", [ROWS, WM], f32, isOutput=False)
    m_in = nc.declare_dram_parameter("m_slab", [ROWS, W], bf16, isOutput=False)
    mi_in = nc.declare_dram_parameter("mi_slab", [ROWS, L, W], bf16, isOutput=False)
    w_in = nc.declare_dram_parameter("wmats", [128, 4, 128], bf16, isOutput=False)
    wdr_in = nc.declare_dram_parameter("wdr", [128, 2, 128], fp8, isOutput=False)
    out = nc.declare_dram_parameter("xout", [L, OUT_ROWS, W], f32, isOutput=True)

    def pool_mul(out_ap, in0, in1):
        nc.gpsimd.tensor_mul(out=out_ap, in0=in0, in1=in1)

    with tile.TileContext(nc) as tc:
        with (
            tc.tile_pool(name="state", bufs=1) as st,
            tc.tile_pool(name="ybps", bufs=1, space="PSUM") as ybp,
            tc.tile_pool(name="cps", bufs=3, space="PSUM") as cp,
        ):
            # ---- load inputs (small ones first; mi streams during preamble)
            y_sb = st.tile([ROWS, WM], f32)
            m_sb = st.tile([ROWS, W], bf16)
            wm = st.tile([128, 4, 128], bf16)
            mi = st.tile([ROWS, L, W], bf16)
            wdr = st.tile([128, 2, 128], fp8)
            nc.sync.dma_start(y_sb[:], y_in[:])
            nc.sync.dma_start(m_sb[:], m_in[:])
            nc.sync.dma_start(wm[:], w_in[:])
            nc.sync.dma_start(wdr[:], wdr_in[:])
            nc.sync.dma_start(mi[:], mi_in[:])

            W_I = wm[:, 0, :]
            W_C = [wm[:, 1 + t, :] for t in range(3)]  # col taps -1, 0, +1

            # ---- persistent state
            ybf = st.tile([ROWS, WM], bf16)
            m2_sb = st.tile([ROWS, W], bf16)
            s_sb = st.tile([ROWS, WM], f32)
            stmp = st.tile([ROWS, WM], f32)
            t0_sb = st.tile([ROWS, WM], bf16)
            xs = st.tile([ROWS, L, XP], bf16)
            zr = st.tile([128, L], bf16)
            # w buffers must be distinct per quad: their conv consumers are
            # emitted a whole loop later, so any slot reuse would make the
            # program-order dep tracker bind those convs to the wrong write
            wq = [st.tile([ROWS, 4, XP], bf16, name=f"wq{i}") for i in range(6)]
            NVW = 5
            vq = [st.tile([ROWS, 4, XP], bf16, name=f"vq{i}") for i in range(NVW)]
            sq = [
                st.tile([ROWS, 4, XP], bf16, name=f"sq{i}")
                for i in range(len(WING_QUADS))
            ]
            up = [st.tile([ROWS, 2, W], bf16, name=f"up{i}") for i in range(NPAIR)]
            # fp8 u pair buffers: band 0 at bytes [0, W), band 1 PRE-SHIFTED
            # one column right at bytes [PW8+1, PW8+1+W); the gap bytes stay
            # zero so DoubleRow reads of either edge contribute nothing
            up8 = [st.tile([ROWS, 2 * PW8], fp8, name=f"up8{i}") for i in range(8)]
            stg = [st.tile([ROWS, 2, W], f32, name=f"stg{i}") for i in range(3)]

            nc.vector.tensor_copy(ybf[:], y_sb[:])
            nc.vector.tensor_mul(out=m2_sb[:], in0=m_sb[:], in1=m_sb[:])
            nc.vector.memset(zr[:], 0.0)
            for t in up8:
                # only the inter-band gap bytes are ever read but not
                # written: view(0, 512) and view(1, 0) = bytes [512, 529)
                nc.vector.memset(t[:, W : PW8 + 1], 0.0)

            def up8_wview(p):
                # skewed write view: (j, c) -> byte j*(PW8+1) + c
                base = up8[p][:, 0:W]
                pr = [list(x) for x in base.ap]
                return AP(base.tensor, base.offset, [pr[0], [PW8 + 1, 2], [1, W]])

            def up8_rview(p, c0, n):
                # DoubleRow rhs view: (j, c) -> byte c0 + j*PW8 + c
                base = up8[p][:, 0:1]
                pr = [list(x) for x in base.ap]
                return AP(
                    base.tensor, base.offset + c0, [pr[0], [PW8, 2], [1, n]]
                )
            zp = st.tile([128, 2], bf16)
            nc.vector.memset(zp[:], 0.0)
            # zero the pad columns once; all later writes stay inside [2, 514)
            for t in (xs, *wq, *vq, *sq):
                nb = t.shape[1]
                nc.vector.tensor_copy(
                    t[:, :, 0:2], zp[:, None, :].to_broadcast((ROWS, nb, 2))
                )
                nc.vector.tensor_copy(
                    t[:, :, XP - 2 : XP], zp[:, None, :].to_broadcast((ROWS, nb, 2))
                )

            yb_tiles = {}

            def yb_tile(k):
                if k not in yb_tiles:
                    yb_tiles[k] = ybp.tile(
                        [ROWS, WM + 5], f32, tag="yb", name=f"yb{k}"
                    )
                return yb_tiles[k]

            def emit_zero_tail(k):
                nc.tensor.matmul(
                    yb_tile(k)[:, W : W + L], W_I, zr[:],
                    start=True, stop=False, skip_group_check=True,
                )

            def emit_A_band(k, l, u_ap, stop=False):
                # matmul outs must not cross the PSUM bank boundary at col 512
                yb = yb_tile(k)
                if l == 0:
                    nc.tensor.matmul(
                        yb[:, 0:W], W_I, u_ap,
                        start=True, stop=False, skip_group_check=True,
                    )
                else:
                    nc.tensor.matmul(
                        yb[:, l:W], W_I, u_ap[:, 0 : W - l],
                        start=False, stop=False, skip_group_check=True,
                    )
                    nc.tensor.matmul(
                        yb[:, W : W + l], W_I, u_ap[:, W - l : W],
                        start=False, stop=stop, skip_group_check=True,
                    )

            # ---- preamble: u0 = (m*m)*y[shift] (one DVE/Pool op per pair),
            # x0 = m*y[shift] (one 28-band DVE op), then the A(0) block
            for p in range(NPAIR):
                if p in (0, 2, 4, 6):
                    pool_mul(
                        up[p][:],
                        m2_sb[:, None, :].to_broadcast((ROWS, 2, W)),
                        _win3(ybf, 2 * p, 2, W),
                    )
                else:
                    nc.vector.tensor_mul(
                        out=up[p][:],
                        in0=m2_sb[:, None, :].to_broadcast((ROWS, 2, W)),
                        in1=_win3(ybf, 2 * p, 2, W),
                    )
                if p % 2 == 1:
                    q = p // 2   # xs quads 0..6 interleaved with the u pairs
                    nc.vector.tensor_mul(
                        out=xs[:, 4 * q : 4 * q + 4, 2 : 2 + W],
                        in0=m_sb[:, None, :].to_broadcast((ROWS, 4, W)),
                        in1=_win3(ybf, 4 * q, 4, W),
                    )
            nc.vector.tensor_scalar_mul(s_sb[:], y_sb[:], 2.0)
            emit_zero_tail(0)
            for p in range(NPAIR):
                emit_A_band(0, 2 * p, up[p][:, 0, :])
                emit_A_band(0, 2 * p + 1, up[p][:, 1, :],
                            stop=(p == NPAIR - 1))

            # ---- iterations
            x2_tiles = {}   # (k, pair) -> (psum tile, band offset)

            def x2_pair(k, j):
                t = cp.tile([ROWS, 2, W], f32, tag="x2", name=f"x2_{k}_{j}")
                x2_tiles[(k, j)] = (t, 0)
                return x2_tiles[(k, j)]

            def emit_conv_mms(x2, rhs_tile, idx, start, stop, sw_tile=None,
                              sw_idx=None):
                # rhs_tile: [ROWS, nb, XP] holding the band at cols [2, 514)
                nc.tensor.matmul(
                    x2, W_C[1], rhs_tile[:, idx, 2 : 2 + W],
                    start=start, stop=False, skip_group_check=True,
                )
                if sw_tile is not None:
                    # wing taps fused: g3[0]*B @ (w<<1 + w>>1)
                    nc.tensor.matmul(
                        x2, W_C[0], sw_tile[:, sw_idx, 2 : 2 + W],
                        start=False, stop=stop, skip_group_check=True,
                    )
                    return
                nc.tensor.matmul(
                    x2, W_C[0], rhs_tile[:, idx, 1 : 1 + W],
                    start=False, stop=False, skip_group_check=True,
                )
                nc.tensor.matmul(
                    x2, W_C[2], rhs_tile[:, idx, 3 : 3 + W],
                    start=False, stop=stop, skip_group_check=True,
                )

            def pair_out_ap(j):
                # xout[2j:2j+2, :, :] with iteration order (row, band, col)
                # to match the [64, 2, W] PSUM source AP
                base = out[2 * j : 2 * j + 2, :, :]
                pr = [list(p) for p in base.ap]
                return AP(base.tensor, base.offset, [pr[1], pr[0], pr[2]])

            def emit_copy_pair(k, j, last):
                t, b0 = x2_tiles[(k, j)]
                x2 = t[:, b0 : b0 + 2, :]
                if last:
                    sg = stg[j % 3]
                    nc.scalar.copy(sg[:], x2)
                    nc.sync.dma_start(
                        pair_out_ap(j), sg[HALO : HALO + OUT_ROWS, :, :]
                    )
                    return
                nc.scalar.copy(xs[:, 2 * j : 2 * j + 2, 2 : 2 + W], x2)

            def emit_u_pair(k, p):
                # u_l = m * x_l (new xs) -> feeds the yb(k+1) A block
                if p in POOL_U_PAIRS:
                    dr = k + 1 <= FP8_LAST_A and p < 8
                    pool_mul(
                        up8_wview(p) if dr else up[p][:],
                        m_sb[:, None, :].to_broadcast((ROWS, 2, W)),
                        xs[:, 2 * p : 2 * p + 2, 2 : 2 + W],
                    )
                else:
                    nc.vector.tensor_mul(
                        out=up[p][:],
                        in0=m_sb[:, None, :].to_broadcast((ROWS, 2, W)),
                        in1=xs[:, 2 * p : 2 * p + 2, 2 : 2 + W],
                    )

            for k in range(n_iter):
                last = k == n_iter - 1
                yb = yb_tile(k)
                # boundary: conv(x) of split bands 0,1 needs no t0 -> PE
                # crosses the A->t0->v0 chain without idling
                t0p, o0p = x2_pair(k, 0)
                for b in range(2):
                    emit_conv_mms(t0p[:, o0p + b, :], xs, b,
                                  start=True, stop=False)
                t1p, o1p = x2_pair(k, 1)
                for b in (2, 3):
                    emit_conv_mms(t1p[:, o1p + b - 2, :], xs, b,
                                  start=True, stop=False)
                # B: t0 = s - 2*yb  (bf16 out)
                nc.vector.scalar_tensor_tensor(
                    out=t0_sb[:], in0=yb[:, 0:WM], scalar=-2.0,
                    in1=s_sb[:], op0=MUL, op1=ADD,
                )
                if not last:
                    # s' = 0.5*(s + t0) + y == s + y - yb, first in the Pool
                    # queue (needs only t0); the 0.5 scale runs on ACT where
                    # a scaled copy is cheap.  The final y-add is deferred
                    # into the conv loop so the cross-engine wait does not
                    # head-of-line block the pool queue.
                    nc.gpsimd.tensor_add(out=stmp[:], in0=s_sb[:], in1=t0_sb[:])
                    nc.scalar.mul(s_sb[:], stmp[:], 0.5)
                # quad 0: v in two halves; conv + copies interleaved
                nc.vector.tensor_mul(
                    out=vq[0][:, 0:2, 2 : 2 + W],
                    in0=mi[:, 0:2, :],
                    in1=_win3(t0_sb, 0, 2, W),
                )
                for b in range(2):
                    emit_conv_mms(t0p[:, o0p + b, :], vq[0], b,
                                  start=False, stop=True)
                emit_copy_pair(k, 0, last)
                nc.vector.tensor_mul(
                    out=vq[0][:, 2:4, 2 : 2 + W],
                    in0=mi[:, 2:4, :],
                    in1=_win3(t0_sb, 2, 2, W),
                )
                for b in (2, 3):
                    emit_conv_mms(t1p[:, o1p + b - 2, :], vq[0], b,
                                  start=False, stop=True)
                emit_copy_pair(k, 1, last)
                # all remaining v/w upfront (sw after each wing quad's w):
                # the in-order DVE queue stays quads ahead of the PE conv loop
                for q in range(1, 7):
                    halves = ((0, 4),)
                    for h0, h1 in halves:
                        nc.vector.tensor_mul(
                            out=vq[q % NVW][:, h0:h1, 2 : 2 + W],
                            in0=mi[:, 4 * q + h0 : 4 * q + h1, :],
                            in1=_win3(t0_sb, 4 * q + h0, h1 - h0, W),
                        )
                        nc.vector.tensor_add(
                            out=wq[q - 1][:, h0:h1, 2 : 2 + W],
                            in0=xs[:, 4 * q + h0 : 4 * q + h1, 2 : 2 + W],
                            in1=vq[q % NVW][:, h0:h1, 2 : 2 + W],
                        )
                    if q in WING_QUADS:
                        # wing sum right after this quad's w lands
                        wb = wq[q - 1]
                        nc.vector.tensor_add(
                            out=sq[WING_QUADS.index(q)][:, 0:4, 2 : 2 + W],
                            in0=wb[:, 0:4, 1 : 1 + W],
                            in1=wb[:, 0:4, 3 : 3 + W],
                        )
                # conv loop (PE) + pair copies (ACT) + Pool u pairs;
                # A(k+1) matmuls interleave in, lagging the u pairs by
                # ~3 quads, so only pairs >= 8 remain after the conv tail
                def emit_A_pair(kk, p, stop=False):
                    if kk <= FP8_LAST_A and p < 8:
                        # fp8 DoubleRow: one matmul accumulates both bands of
                        # the pre-shifted pair; split at the PSUM bank edge
                        yb = yb_tile(kk)
                        l0 = 2 * p
                        n1 = W - l0
                        nc.tensor.matmul(
                            yb[:, l0:W], wdr[:], up8_rview(p, 0, n1),
                            start=(p == 0), stop=False, perf_mode=DRMODE,
                            skip_group_check=True,
                        )
                        nc.tensor.matmul(
                            yb[:, W : W + l0 + 1], wdr[:],
                            up8_rview(p, n1, l0 + 1),
                            start=False, stop=stop, perf_mode=DRMODE,
                            skip_group_check=True,
                        )
                        return
                    emit_A_band(kk, 2 * p, up[p][:, 0, :])
                    emit_A_band(kk, 2 * p + 1, up[p][:, 1, :], stop=stop)

                for idx, q in enumerate(range(1, 7)):
                    wing = q in WING_QUADS
                    wb = wq[q - 1]
                    sb = sq[WING_QUADS.index(q)] if wing else None
                    for jj in range(2):
                        tj, oj = x2_pair(k, 2 * q + jj)
                        for bb in range(2):
                            b = 4 * q + 2 * jj + bb
                            emit_conv_mms(
                                tj[:, oj + bb, :], wb, b - 4 * q,
                                start=True, stop=True,
                                sw_tile=sb, sw_idx=(b - 4 * q) if wing else None,
                            )
                        emit_copy_pair(k, 2 * q + jj, last)
                    if not last:
                        if q == 2:
                            # deferred tail of the s update (ACT scale done)
                            nc.gpsimd.tensor_add(
                                out=s_sb[:], in0=s_sb[:], in1=y_sb[:]
                            )
                        if q <= 4:
                            # Pool u pairs, spread so GPSIMD starts early
                            for p in (2 * q - 2, 2 * q - 1):
                                if p in POOL_U_PAIRS:
                                    emit_u_pair(k, p)
                        # A(k+1) pairs interleave, lagging the pool u pairs
                        if idx == 2:
                            emit_zero_tail(k + 1)
                        if idx >= 2:
                            emit_A_pair(k + 1, 2 * (idx - 2))
                            emit_A_pair(k + 1, 2 * (idx - 2) + 1)
                if not last:
                    # remaining u pairs on DVE; pair 13 is computed BEFORE
                    # pair 12 so the A group's stop matmul (pair 12, emitted
                    # last) does not wait on DVE's final op
                    for p in (8, 9, 10, 11, 13, 12):
                        if p not in POOL_U_PAIRS:
                            emit_u_pair(k, p)
                    for p in (8, 9, 10, 11, 13):
                        emit_A_pair(k + 1, p)
                    emit_A_pair(k + 1, 12, stop=True)

    _split_excess_waits(nc, max_w=1)
    return nc


def _host_inputs(y_1hw, mask2d):
    y2 = np.asarray(y_1hw, dtype=np.float32)[0]      # [512, 539]
    m2 = np.asarray(mask2d, dtype=np.float32)        # [512, 512]
    g3 = _gauss3(SIGMA)

    # Phi_sum / mi on the full grid (host precompute; Phi depends only on m)
    Phi = np.zeros((H, WM), dtype=np.float32)
    for l in range(L):
        Phi[:, l : l + W] += m2
    Phi = np.maximum(Phi, 1.0)
    invPhi = (1.0 / Phi).astype(np.float32)

    ident = np.eye(128, dtype=np.float32)

    in_maps = []
    for c in range(NCORES):
        rk = 64 * c - HALO
        y_slab = np.zeros((ROWS, WM), dtype=np.float32)
        m_slab = np.zeros((ROWS, W), dtype=np.float32)
        mi_slab = np.zeros((ROWS, L, W), dtype=np.float32)
        lo = max(0, -rk)              # first valid slab row
        hi = min(ROWS, H - rk)        # one past last valid slab row
        y_slab[lo:hi] = y2[rk + lo : rk + hi]
        m_slab[lo:hi] = m2[rk + lo : rk + hi]
        iv = invPhi[rk + lo : rk + hi]  # [vr, WM]
        for l in range(L):
            mi_slab[lo:hi, l, :] = m_slab[lo:hi] * iv[:, l : l + W]
        # banded 3-tap row-conv matrix, zeroed outside the valid row range
        B = np.zeros((128, 128), dtype=np.float32)
        for kk in range(-1, 2):
            for i in range(128):
                ip = i + kk
                if lo <= i < hi and lo <= ip < hi:
                    B[ip, i] = g3[kk + 1]
        wmats = np.zeros((128, 4, 128), dtype=np.float32)
        wmats[:, 0, :] = ident
        for t in range(3):
            wmats[:, 1 + t, :] = g3[t] * B
        wdr = np.zeros((128, 2, 128), dtype=np.float32)
        wdr[:, 0, :] = ident
        wdr[:, 1, :] = ident
        in_maps.append(
            {
                "y_slab": y_slab,
                "m_slab": m_slab.astype(ml_dtypes.bfloat16),
                "mi_slab": mi_slab.astype(ml_dtypes.bfloat16),
                "wmats": wmats.astype(ml_dtypes.bfloat16),
                "wdr": wdr.astype(ml_dtypes.float8_e4m3fn),
            }
        )
    return in_maps


_NC_CACHE = {}


def _get_nc(dx, n_iter=N_ITER):
    key = (tuple(int(v) for v in dx), n_iter)
    if key not in _NC_CACHE:
        assert all(int(d) == i for i, d in enumerate(key[0])), (
            "kernel assumes dx[l] == l"
        )
        _NC_CACHE[key] = build_nc(n_iter)
    return _NC_CACHE[key]


def kernel(y_1hw, mask2d, phi_d_deg, s_nom, n_iter=N_ITER, trace=False):
    s = np.asarray(s_nom, dtype=np.float32)
    phi = float(np.asarray(phi_d_deg))
    dx, dy = _offsets(s, phi)
    assert (dy == 0).all(), "kernel assumes dy == 0 (row shifts unsupported)"
    nc = _get_nc(dx, n_iter)
    in_maps = _host_inputs(y_1hw, mask2d)
    res = run_bass_kernel_spmd(nc, in_maps, list(range(NCORES)), trace=trace)
    x_full = np.empty((1, L, H, W), dtype=np.float32)
    for c in range(NCORES):
        x_full[0, :, 64 * c : 64 * (c + 1), :] = res.results[c]["xout"]
    kernel.last_results = res
    return x_full
